# revision 1
# baseline (speedup 1.0000x reference)
"""GCNConv (rank-1 normalized aggregation) Trainium2 kernel, SPMD over 8 cores.

Math (faithful to the torch/jax reference):
    h    = x @ W
    adj  = symmetric 0/1 adjacency from edge_index (duplicates collapse: SET, not add)
    deg  = adj.sum(1);  dinv = 1/sqrt(deg)
    agg  = dinv @ h                      # rank-1 identity, [F_OUT]
    out  = dinv[:, None] * agg[None, :] + bias

Since agg = (dinv @ x) @ W, h is never materialized:
    v    = dinv @ x            ([F_IN] weighted row-sum, DVE mul + strided reduce)
    agg  = v @ W               (TensorE)
    out_c = dinv_c (x) agg + bias     (rows sharded across cores)

Collectives in this environment have a ~55us fixed latency (measured with a
bare 512B AllReduce), far above the 8-core floor, so instead of sharding the
v-reduction + AllReduce, every core reads the full x (6.1MB, ~17us at HBM BW)
and computes v locally; only the O(N*F_OUT) output is sharded.

The exact deduplicated degree (an integer/sorting problem, not a flops
problem) is computed on host with np.unique; all O(N*F) floating-point work
runs on the NeuronCores.
"""

import numpy as np

N, F_IN, F_OUT = 12000, 128, 256
N_CORES = 8
ROWS = N // N_CORES            # 1500 output rows per core
NT_OUT = 12                    # 12 row tiles per core (padded)
ROWS_PAD = NT_OUT * 128        # 1536
NT_FULL = 96                   # full-x row tiles (padded)
N_PAD = NT_FULL * 128          # 12288
# x rows-per-partition per DMA/compute chunk; small first chunks so DVE
# starts sooner, ramping up once the pipeline is primed
CHUNK_SIZES = [8, 8, 12, 12, 16, 16, 12, 12]
N_CHUNKS = len(CHUNK_SIZES)

_cache = {}


def _build_nc():
    import concourse.bacc as bacc
    import concourse.mybir as mybir
    import concourse.tile as tile

    f32 = mybir.dt.float32
    bf16 = mybir.dt.bfloat16

    nc = bacc.Bacc(
        "TRN2",
        target_bir_lowering=False,
        debug=False,
        num_devices=N_CORES,
    )

    # x and dinvT travel as bf16: halves DMA bytes and DVE mul time; the
    # ~0.3% relative error on v is far inside the 2e-2 gate
    x_d = nc.dram_tensor("x", [N_PAD, F_IN], bf16, kind="ExternalInput")
    # dinvT[p, r] = dinv[p*96 + r] (host-prepared layout matching x view)
    dinvT_d = nc.dram_tensor("dinvT", [128, NT_FULL], bf16, kind="ExternalInput")
    # f32 copy for the ScalarE activation scale operand
    dinvTf_d = nc.dram_tensor("dinvTf", [128, NT_FULL], f32, kind="ExternalInput")
    dinvS_d = nc.dram_tensor("dinvS", [128, NT_OUT], f32, kind="ExternalInput")
    w_d = nc.dram_tensor("weight", [F_IN, F_OUT], bf16, kind="ExternalInput")
    b_d = nc.dram_tensor("bias", [F_OUT], f32, kind="ExternalInput")
    out_d = nc.dram_tensor("out", [ROWS_PAD, F_OUT], f32, kind="ExternalOutput")

    # x view: partition p holds rows [p*96, (p+1)*96) -> one contiguous 48KB
    # read per partition (vs 2048 scattered 512B runs for the (n p) m view)
    x_prm = x_d.ap().rearrange("(p r) m -> p r m", p=128)      # [128,96,128]
    out_pnm = out_d.ap().rearrange("(n p) m -> p n m", p=128)  # [128,12,256]

    dma_engines = [nc.sync, nc.scalar]

    with tile.TileContext(nc) as tc:
        with (
            tc.tile_pool(name="const", bufs=1) as cpool,
            tc.tile_pool(name="xbuf", bufs=1) as xpool,
            tc.tile_pool(name="scl", bufs=3) as spool,
            tc.tile_pool(name="obuf", bufs=1) as opool,
            tc.tile_pool(name="ps", bufs=1, space="PSUM") as psum,
        ):
            # ---- small constants first (cheap), then x chunks ----
            # (keep everything off gpsimd: SWDGE completion latency is ~9us
            # and its drain blocks dependents)
            dinvT = cpool.tile([128, NT_FULL], bf16)
            nc.sync.dma_start(dinvT[:], dinvT_d.ap())
            dinvTf = cpool.tile([128, NT_FULL], f32)
            nc.scalar.dma_start(dinvTf[:], dinvTf_d.ap())
            bias_s = cpool.tile([1, F_OUT], f32)
            nc.scalar.dma_start(bias_s[:], b_d.ap().rearrange("(a n) -> a n", a=1))

            xc = []
            off = 0
            offs = []
            for q in range(N_CHUNKS):
                sz = CHUNK_SIZES[q]
                t = xpool.tile([128, sz, F_IN], bf16, tag=f"xc{q}", name=f"xc{q}")
                dma_engines[q % len(dma_engines)].dma_start(
                    t[:], x_prm[:, off : off + sz, :]
                )
                xc.append(t)
                offs.append(off)
                off += sz

            # needed only mid/late kernel; queue after the x chunks
            dinvS = cpool.tile([128, NT_OUT], f32)
            nc.scalar.dma_start(dinvS[:], dinvS_d.ap())
            w_s = cpool.tile([F_IN, F_OUT], bf16)
            nc.sync.dma_start(w_s[:], w_d.ap())

            ones_col = cpool.tile([128, 1], bf16)
            nc.vector.memset(ones_col[:], 1.0)
            ones_row = cpool.tile([1, 128], f32)
            nc.vector.memset(ones_row[:], 1.0)

            # ---- v = dinv @ x ----
            # per chunk: scaled = x * dinv (DVE); TensorE contracts partitions
            # via ones-matmuls, ALL accumulating into one [1,512] PSUM bank:
            # pvw[0, u] = sum over rows r with r%4 == u//128 of dinv_r*x[r, u%128]
            pvw = psum.tile([1, 512], f32)
            total_sl = sum(CHUNK_SIZES) * F_IN // 512
            sl = 0
            for q in range(N_CHUNKS):
                sz = CHUNK_SIZES[q]
                d_bc = (
                    dinvT[:, offs[q] : offs[q] + sz]
                    .unsqueeze(2)
                    .broadcast_to([128, sz, F_IN])
                )
                scaled = spool.tile([128, sz, F_IN], bf16, tag=f"scaled{q % 3}",
                                    name=f"scaled{q}")
                if q >= N_CHUNKS - 6:
                    # late chunks: split the scaling DVE/ScalarE so the
                    # pipeline tail shortens (ACT does the last 4 rows;
                    # by then the Activation sequencer has issued all DMAs)
                    dv = sz - 4
                    nc.vector.tensor_mul(
                        scaled[:, :dv, :], xc[q][:, :dv, :],
                        d_bc[:, :dv, :],
                    )
                    for r in range(dv, sz):
                        nc.scalar.activation(
                            scaled[:, r, :],
                            xc[q][:, r, :],
                            mybir.ActivationFunctionType.Copy,
                            scale=dinvTf[:, offs[q] + r : offs[q] + r + 1],
                        )
                else:
                    nc.vector.tensor_mul(scaled[:], xc[q][:], d_bc)
                flat = scaled[:].rearrange("p t j -> p (t j)")
                for s in range((sz * F_IN) // 512):
                    nc.tensor.matmul(
                        pvw[:],
                        ones_col[:],
                        flat[:, s * 512 : (s + 1) * 512],
                        start=(sl == 0),
                        stop=(sl == total_sl - 1),
                        skip_group_check=True,
                    )
                    sl += 1
            # fold the 4 t-mod groups: one small strided reduce
            vrow = cpool.tile([1, F_IN], f32)
            nc.vector.tensor_reduce(
                vrow[:],
                pvw[:].rearrange("a (t j) -> a j t", j=F_IN),
                axis=mybir.AxisListType.X,
                op=mybir.AluOpType.add,
            )

            # v [1,128] -> vcol [128,1] via TensorE transpose; cast to bf16
            # (for the A2 matmul whose rhs W is bf16) in the PSUM->SBUF copy
            pvcol = psum.tile([F_IN, 1], f32)
            nc.tensor.transpose(pvcol[:], vrow[:], ones_row[:1, :1])
            vcol = cpool.tile([F_IN, 1], bf16)
            nc.vector.tensor_copy(vcol[:], pvcol[:])

            # ---- A2[p, o] = agg[o] = sum_j v[j] W[j, o]  (v bcast as lhsT) ----
            pA2 = psum.tile([128, F_OUT], f32)
            nc.tensor.matmul(
                pA2[:],
                vcol[:].broadcast_to([F_IN, 128]),
                w_s[:],
                start=True,
                stop=True,
            )
            A2 = cpool.tile([128, F_OUT], f32)
            nc.vector.tensor_copy(A2[:], pA2[:])
            pB2 = psum.tile([128, F_OUT], f32)
            nc.tensor.matmul(pB2[:], ones_row[:], bias_s[:], start=True, stop=True)
            B2 = cpool.tile([128, F_OUT], f32)
            nc.vector.tensor_copy(B2[:], pB2[:])

            # ---- out tile i = (A2 * dinvS_i) + B2, one fused DVE op each ----
            # shrinking DMA groups so the last transfer is small
            out_engines = [nc.sync, nc.scalar]
            og_sizes = [3, 3, 2, 2, 1, 1]
            base = 0
            for g, gsz in enumerate(og_sizes):
                og = opool.tile([128, gsz, F_OUT], f32, tag=f"og{g}",
                                name=f"og{g}")
                for j in range(gsz):
                    i = base + j
                    nc.vector.scalar_tensor_tensor(
                        og[:, j, :],
                        A2[:],
                        dinvS[:, i : i + 1],
                        B2[:],
                        op0=mybir.AluOpType.mult,
                        op1=mybir.AluOpType.add,
                    )
                out_engines[g % 2].dma_start(
                    out_pnm[:, base : base + gsz, :], og[:]
                )
                base += gsz

    nc.compile()
    return nc


def _get_nc():
    if "nc" not in _cache:
        _cache["nc"] = _build_nc()
    return _cache["nc"]


def _host_dinv(edge_index: np.ndarray) -> np.ndarray:
    """Exact deduplicated symmetric degree -> 1/sqrt(deg), matching
    adj[a,b]=1; adj[b,a]=1; deg=adj.sum(1)."""
    a = edge_index[0].astype(np.int64)
    b = edge_index[1].astype(np.int64)
    keys = np.unique(np.concatenate([a * N + b, b * N + a]))
    deg = np.bincount(keys // N, minlength=N).astype(np.float32)
    with np.errstate(divide="ignore"):
        dinv = (np.float32(1.0) / np.sqrt(deg)).astype(np.float32)
    return dinv


def kernel(x, edge_index, weight, bias, _trace=False):
    from concourse import bass_utils

    x = np.ascontiguousarray(x, dtype=np.float32)
    weight = np.ascontiguousarray(weight, dtype=np.float32)
    bias = np.ascontiguousarray(bias, dtype=np.float32)
    dinv = _host_dinv(np.asarray(edge_index))

    nc = _get_nc()

    import ml_dtypes

    bf16 = ml_dtypes.bfloat16
    xp = np.zeros((N_PAD, F_IN), bf16)
    xp[:N] = x.astype(bf16)
    dp = np.zeros((N_PAD,), np.float32)
    dp[:N] = dinv
    # dinvT[p, r] = dinv[p*96 + r], matching the x view "(p r) m -> p r m"
    dinvTf = np.ascontiguousarray(dp.reshape(128, NT_FULL))
    dinvT = dinvTf.astype(bf16)

    w16 = weight.astype(bf16)
    in_maps = []
    for c in range(N_CORES):
        r0 = c * ROWS
        ds = np.zeros((ROWS_PAD,), np.float32)
        ds[:ROWS] = dinv[r0 : r0 + ROWS]
        dinvS = np.ascontiguousarray(ds.reshape(NT_OUT, 128).T)  # [128, 12]
        in_maps.append(
            {
                "x": xp,
                "dinvT": dinvT,
                "dinvTf": dinvTf,
                "dinvS": dinvS,
                "weight": w16,
                "bias": bias,
            }
        )

    res = bass_utils.run_bass_kernel_spmd(
        nc, in_maps, core_ids=list(range(N_CORES)), trace=_trace
    )
    out = np.concatenate(
        [res.results[c]["out"][:ROWS] for c in range(N_CORES)], axis=0
    )
    if _trace:
        _cache["last_results"] = res
    return out



# revision 4
# speedup vs baseline: 1.1263x; 1.1263x over previous
"""GCNConv (rank-1 normalized aggregation) Trainium2 kernel, SPMD over 8 cores.

Math (faithful to the torch/jax reference):
    h    = x @ W
    adj  = symmetric 0/1 adjacency from edge_index (duplicates collapse: SET, not add)
    deg  = adj.sum(1);  dinv = 1/sqrt(deg)
    agg  = dinv @ h                      # rank-1 identity, [F_OUT]
    out  = dinv[:, None] * agg[None, :] + bias

Since agg = (dinv @ x) @ W, h is never materialized:
    v    = dinv @ x            ([F_IN] weighted row-sum)
    agg  = v @ W               (TensorE)
    out_c = dinv_c (x) agg + bias     (rows sharded across cores)

Collectives here have a ~55us fixed latency, far above the 8-core floor, so
every core reads the full x (3.07MB bf16, ~9us at HBM BW) and computes v
locally; only the O(N*F_OUT) output is sharded.

v is computed entirely on TensorE: for each 128-node chunk, the dinv slice
[128,1] is the stationary operand (LDWEIGHTS cost scales with stationary
*columns*, so a 1-column load is ~1 cycle) and the raw bf16 x chunk [128,128]
is the moving operand; all 96 matmuls accumulate into one [1,128] PSUM tile.
This removes the DVE pre-multiply (6.4us at the 2x DVE ceiling) and the
fat ones-matmul rhs streaming (20k+ columns) of the previous version.

The exact deduplicated degree (an integer/sorting problem, not a flops
problem) is computed on host with np.unique; all O(N*F) floating-point work
runs on the NeuronCores. Output travels bf16 and is upcast on host.
"""

import numpy as np

N, F_IN, F_OUT = 12000, 128, 256
N_CORES = 8
ROWS = N // N_CORES            # 1500 output rows per core
NT_OUT = 12                    # rows per partition in the output shard
ROWS_PAD = NT_OUT * 128        # 1536
R_PER_P = 8                    # x rows per partition per chunk (2KB DMA runs)
N_CHUNKS = 12                  # 12 chunks x 1024 nodes
CHUNK_NODES = 128 * R_PER_P    # 1024
N_PAD = N_CHUNKS * CHUNK_NODES # 12288
QCOLS = N_CHUNKS * R_PER_P     # 96 dinv stationary columns
OUT_GROUPS = 4                 # expansion/write pipeline groups
NT_G = NT_OUT // OUT_GROUPS    # 3 rows per partition per group

_cache = {}


def _build_nc(zero_bias):
    import concourse.bacc as bacc
    import concourse.mybir as mybir
    import concourse.tile as tile

    f32 = mybir.dt.float32
    bf16 = mybir.dt.bfloat16

    nc = bacc.Bacc(
        "TRN2",
        target_bir_lowering=False,
        debug=False,
        num_devices=N_CORES,
    )

    # x padded to [12288, 128] bf16, natural row-major.
    x_d = nc.dram_tensor("x", [N_PAD, F_IN], bf16, kind="ExternalInput")
    # dinvQ[p, c*8+r] = dinv[c*1024 + p*8 + r]  (matches the x chunk view)
    dinvQ_d = nc.dram_tensor("dinvQ", [128, QCOLS], bf16, kind="ExternalInput")
    # dinvS[p, i] = dinv[core*1500 + p*12 + i]  (output shard scalars)
    dinvS_d = nc.dram_tensor("dinvS", [128, NT_OUT], bf16, kind="ExternalInput")
    w_d = nc.dram_tensor("weight", [F_IN, F_OUT], bf16, kind="ExternalInput")
    if not zero_bias:
        b_d = nc.dram_tensor("bias", [F_OUT], f32, kind="ExternalInput")
    out_d = nc.dram_tensor("out", [ROWS_PAD, F_OUT], bf16, kind="ExternalOutput")

    # chunk c, partition p holds rows c*1024 + p*8 .. +7 -> 2KB contiguous runs
    x_view = x_d.ap().rearrange("(c p r) m -> p c (r m)", c=N_CHUNKS, p=128)
    # out: partition p holds rows p*12 .. p*12+11 -> row-major natural order
    out_view = out_d.ap().rearrange("(p n) m -> p n m", p=128)

    with tile.TileContext(nc) as tc:
        with (
            tc.tile_pool(name="const", bufs=1) as cpool,
            tc.tile_pool(name="xbuf", bufs=1) as xpool,
            tc.tile_pool(name="obuf", bufs=1) as opool,
            tc.tile_pool(name="ps", bufs=1, space="PSUM") as psum,
        ):
            # ---- DMAs: x chunks stream on the sync queue; everything the
            # tail needs rides the scalar queue concurrently ----
            dinvQ = cpool.tile([128, QCOLS], bf16)
            nc.scalar.dma_start(dinvQ[:], dinvQ_d.ap())

            xc = []
            for c in range(N_CHUNKS):
                t = xpool.tile([128, CHUNK_NODES], bf16, tag=f"xc{c}",
                               name=f"xc{c}")
                nc.sync.dma_start(t[:], x_view[:, c, :])
                xc.append(t)

            w_s = cpool.tile([F_IN, F_OUT], bf16)
            nc.scalar.dma_start(w_s[:], w_d.ap())
            dinvS = cpool.tile([128, NT_OUT], bf16)
            nc.scalar.dma_start(dinvS[:], dinvS_d.ap())
            if not zero_bias:
                bias_s = cpool.tile([1, F_OUT], f32)
                nc.scalar.dma_start(
                    bias_s[:], b_d.ap().rearrange("(a n) -> a n", a=1)
                )
                ones_row = cpool.tile([1, 128], f32)
                nc.vector.memset(ones_row[:], 1.0)
            one_s = cpool.tile([1, 1], f32)
            nc.vector.memset(one_s[:], 1.0)

            # ---- v = dinv @ x : 96 accumulating matmuls, dinv stationary ----
            pv = psum.tile([1, F_IN], f32)
            total = N_CHUNKS * R_PER_P
            q = 0
            for c in range(N_CHUNKS):
                for r in range(R_PER_P):
                    nc.tensor.matmul(
                        pv[:],
                        dinvQ[:, q : q + 1],
                        xc[c][:, r * F_IN : (r + 1) * F_IN],
                        start=(q == 0),
                        stop=(q == total - 1),
                        skip_group_check=True,
                    )
                    q += 1

            # v row -> column via TensorE transpose, cast bf16 for the agg mm
            vrow = cpool.tile([1, F_IN], f32)
            nc.vector.tensor_copy(vrow[:], pv[:])
            pvcol = psum.tile([F_IN, 1], f32)
            nc.tensor.transpose(pvcol[:], vrow[:], one_s[:])
            vcol = cpool.tile([F_IN, 1], bf16)
            nc.vector.tensor_copy(vcol[:], pvcol[:])

            # A2[p, o] = agg[o] = sum_j v[j] W[j, o]   (v bcast as lhsT)
            pA2 = psum.tile([128, F_OUT], f32)
            nc.tensor.matmul(
                pA2[:],
                vcol[:].broadcast_to([F_IN, 128]),
                w_s[:],
                start=True,
                stop=True,
            )
            A2 = cpool.tile([128, F_OUT], bf16)
            nc.vector.tensor_copy(A2[:], pA2[:])
            if not zero_bias:
                pB2 = psum.tile([128, F_OUT], f32)
                nc.tensor.matmul(
                    pB2[:], ones_row[:], bias_s[:], start=True, stop=True
                )
                B2 = cpool.tile([128, F_OUT], bf16)
                nc.vector.tensor_copy(B2[:], pB2[:])

            # ---- out[p, n, :] = dinvS[p, n] * A2 (+ bias), pipelined ----
            A2_bc = A2[:].unsqueeze(1).broadcast_to([128, NT_G, F_OUT])
            for g in range(OUT_GROUPS):
                og = opool.tile([128, NT_G, F_OUT], bf16, tag=f"og{g}",
                                name=f"og{g}")
                d_bc = (
                    dinvS[:, g * NT_G : (g + 1) * NT_G]
                    .unsqueeze(2)
                    .broadcast_to([128, NT_G, F_OUT])
                )
                nc.vector.tensor_mul(og[:], A2_bc, d_bc)
                if not zero_bias:
                    b_bc = B2[:].unsqueeze(1).broadcast_to([128, NT_G, F_OUT])
                    nc.vector.tensor_add(og[:], og[:], b_bc)
                nc.scalar.dma_start(
                    out_view[:, g * NT_G : (g + 1) * NT_G, :], og[:]
                )

    nc.compile()
    return nc


def _get_nc(zero_bias):
    key = ("nc", zero_bias)
    if key not in _cache:
        _cache[key] = _build_nc(zero_bias)
    return _cache[key]


def _host_dinv(edge_index: np.ndarray) -> np.ndarray:
    """Exact deduplicated symmetric degree -> 1/sqrt(deg), matching
    adj[a,b]=1; adj[b,a]=1; deg=adj.sum(1)."""
    a = edge_index[0].astype(np.int64)
    b = edge_index[1].astype(np.int64)
    keys = np.unique(np.concatenate([a * N + b, b * N + a]))
    deg = np.bincount(keys // N, minlength=N).astype(np.float32)
    with np.errstate(divide="ignore"):
        dinv = (np.float32(1.0) / np.sqrt(deg)).astype(np.float32)
    return dinv


def kernel(x, edge_index, weight, bias, _trace=False):
    from concourse import bass_utils

    x = np.ascontiguousarray(x, dtype=np.float32)
    weight = np.ascontiguousarray(weight, dtype=np.float32)
    bias = np.ascontiguousarray(bias, dtype=np.float32)
    dinv = _host_dinv(np.asarray(edge_index))

    zero_bias = bool(np.all(bias == 0.0))
    nc = _get_nc(zero_bias)

    import ml_dtypes

    bf16 = ml_dtypes.bfloat16
    xp = np.zeros((N_PAD, F_IN), bf16)
    xp[:N] = x.astype(bf16)
    dp = np.zeros((N_PAD,), np.float32)
    dp[:N] = dinv
    # dinvQ[p, c*8+r] = dinv[c*1024 + p*8 + r]
    dinvQ = np.ascontiguousarray(
        dp.reshape(N_CHUNKS, 128, R_PER_P).transpose(1, 0, 2).reshape(128, QCOLS)
    ).astype(bf16)

    w16 = weight.astype(bf16)
    in_maps = []
    for c in range(N_CORES):
        r0 = c * ROWS
        ds = np.zeros((ROWS_PAD,), np.float32)
        ds[:ROWS] = dinv[r0 : r0 + ROWS]
        # dinvS[p, i] = dinv[r0 + p*12 + i]
        dinvS = np.ascontiguousarray(ds.reshape(128, NT_OUT)).astype(bf16)
        im = {
            "x": xp,
            "dinvQ": dinvQ,
            "dinvS": dinvS,
            "weight": w16,
        }
        if not zero_bias:
            im["bias"] = bias
        in_maps.append(im)

    res = bass_utils.run_bass_kernel_spmd(
        nc, in_maps, core_ids=list(range(N_CORES)), trace=_trace
    )
    out = np.concatenate(
        [
            res.results[c]["out"][:ROWS].astype(np.float32)
            for c in range(N_CORES)
        ],
        axis=0,
    )
    if _trace:
        _cache["last_results"] = res
    return out


# revision 6
# speedup vs baseline: 1.2789x; 1.1354x over previous
"""GCNConv (rank-1 normalized aggregation) Trainium2 kernel, SPMD over 8 cores.

Math (faithful to the torch/jax reference):
    h    = x @ W
    adj  = symmetric 0/1 adjacency from edge_index (duplicates collapse: SET, not add)
    deg  = adj.sum(1);  dinv = 1/sqrt(deg)
    agg  = dinv @ h                      # rank-1 identity, [F_OUT]
    out  = dinv[:, None] * agg[None, :] + bias

Since agg = (dinv @ x) @ W, h is never materialized:
    v    = dinv @ x            ([F_IN] weighted row-sum)
    agg  = v @ W               (TensorE)
    out_c = dinv_c (x) agg + bias     (rows sharded across cores)

Collectives here have a ~55us fixed latency, far above the 8-core floor, so
every core reads the full x (3.07MB bf16, ~9us at HBM BW) and computes v
locally; only the O(N*F_OUT) output is sharded.

v runs entirely on TensorE: per 128-node slice r, the dinv column [128,1] is
the stationary operand (LDWEIGHTS cost scales with stationary *columns*, so a
1-column load is ~1 cycle) and the raw bf16 x slice [128,128] is the moving
operand; all 96 matmuls accumulate into one [1,128] PSUM tile. No DVE
pre-multiply, no fat ones-matmul.

DMA shape discipline (measured): each 128-descriptor dma_start costs ~680ns
of HWDGE sequencer issue time, so x travels in 6 big r-grouped transfers
(24KB contiguous per partition total) alternating between the two HWDGE
queues, small first group (early TensorE start) and small last group (short
completion-receipt tail). Small constants are packed into one tensor.

The exact deduplicated degree (an integer/sorting problem, not a flops
problem) is computed on host with np.unique; all O(N*F) floating-point work
runs on the NeuronCores. Output travels bf16 and is upcast on host.
"""

import numpy as np

N, F_IN, F_OUT = 12000, 128, 256
N_CORES = 8
ROWS = N // N_CORES            # 1500 output rows per core
NT_OUT = 12                    # rows per partition in the output shard
ROWS_PAD = NT_OUT * 128        # 1536
R_TOT = 96                     # x rows per partition (node = p*96 + r)
N_PAD = 128 * R_TOT            # 12288
X_GROUPS = [8, 12, 20, 28, 20, 8]
DVE_ROWS = 8                   # expansion rows on DVE (tensor_scalar_mul)
ACT_ROWS = NT_OUT - DVE_ROWS   # expansion rows on ScalarE (activation scale)
C_DT = R_TOT + NT_OUT          # packed bf16 consts: dinvT | dinvS

_cache = {}


def _build_nc(zero_bias):
    import concourse.bacc as bacc
    import concourse.mybir as mybir
    import concourse.tile as tile

    f32 = mybir.dt.float32
    bf16 = mybir.dt.bfloat16

    nc = bacc.Bacc(
        "TRN2",
        target_bir_lowering=False,
        debug=False,
        num_devices=N_CORES,
    )

    # x padded to [12288, 128] bf16; partition p holds rows p*96 .. p*96+95
    x_d = nc.dram_tensor("x", [N_PAD, F_IN], bf16, kind="ExternalInput")
    # cb16[:, 0:96] = dinvT (dinvT[p, r] = dinv[p*96+r]);
    # cb16[:, 96:108] = dinvS (dinvS[p, i] = dinv[core*1500 + p*12 + i])
    cb16_d = nc.dram_tensor("cb16", [128, C_DT], bf16, kind="ExternalInput")
    # f32 copy of dinvS for the ScalarE activation scale operand
    dinvSf_d = nc.dram_tensor("dinvSf", [128, NT_OUT], f32, kind="ExternalInput")
    w_d = nc.dram_tensor("weight", [F_IN, F_OUT], bf16, kind="ExternalInput")
    if not zero_bias:
        b_d = nc.dram_tensor("bias", [F_OUT], f32, kind="ExternalInput")
    out_d = nc.dram_tensor("out", [ROWS_PAD, F_OUT], bf16, kind="ExternalOutput")

    x_prm = x_d.ap().rearrange("(p r) m -> p r m", p=128)       # [128,96,128]
    # out row p*12 + n  ->  partition p, free n  (natural row-major)
    out_view = out_d.ap().rearrange("(p n) m -> p n m", p=128)  # [128,12,256]

    with tile.TileContext(nc) as tc:
        with (
            tc.tile_pool(name="const", bufs=1) as cpool,
            tc.tile_pool(name="xbuf", bufs=1) as xpool,
            tc.tile_pool(name="obuf", bufs=1) as opool,
            tc.tile_pool(name="ps", bufs=1, space="PSUM") as psum,
        ):
            # ---- DMAs: x groups alternate the two HWDGE queues; the packed
            # consts lead the scalar queue so TensorE can start on group 0 ----
            cb16 = cpool.tile([128, C_DT], bf16)
            nc.scalar.dma_start(cb16[:], cb16_d.ap())
            dinvT = cb16[:, 0:R_TOT]
            dinvS = cb16[:, R_TOT : R_TOT + NT_OUT]

            xg = []
            r0 = 0
            x_offs = []
            for g, rsz in enumerate(X_GROUPS):
                t = xpool.tile([128, rsz, F_IN], bf16, tag=f"xg{g}",
                               name=f"xg{g}")
                eng = nc.sync if g % 2 == 0 else nc.scalar
                eng.dma_start(t[:], x_prm[:, r0 : r0 + rsz, :])
                xg.append(t)
                x_offs.append(r0)
                r0 += rsz

            dinvSf = cpool.tile([128, NT_OUT], f32)
            nc.scalar.dma_start(dinvSf[:], dinvSf_d.ap())
            w_s = cpool.tile([F_IN, F_OUT], bf16)
            nc.scalar.dma_start(w_s[:], w_d.ap())
            if not zero_bias:
                bias_s = cpool.tile([1, F_OUT], f32)
                nc.scalar.dma_start(
                    bias_s[:], b_d.ap().rearrange("(a n) -> a n", a=1)
                )
                ones_row = cpool.tile([1, 128], f32)
                nc.vector.memset(ones_row[:], 1.0)
            one_s = cpool.tile([1, 1], f32)
            nc.vector.memset(one_s[:], 1.0)

            # ---- v = dinv @ x : 96 accumulating matmuls, dinv stationary ----
            pv = psum.tile([1, F_IN], f32)
            q = 0
            for g, rsz in enumerate(X_GROUPS):
                for r in range(rsz):
                    nc.tensor.matmul(
                        pv[:],
                        dinvT[:, x_offs[g] + r : x_offs[g] + r + 1],
                        xg[g][:, r, :],
                        start=(q == 0),
                        stop=(q == R_TOT - 1),
                        skip_group_check=True,
                    )
                    q += 1

            # v row -> column via TensorE transpose, cast bf16 for the agg mm
            vrow = cpool.tile([1, F_IN], f32)
            nc.vector.tensor_copy(vrow[:], pv[:])
            pvcol = psum.tile([F_IN, 1], f32)
            nc.tensor.transpose(pvcol[:], vrow[:], one_s[:])
            vcol = cpool.tile([F_IN, 1], bf16)
            nc.vector.tensor_copy(vcol[:], pvcol[:])

            # A2[p, o] = agg[o] = sum_j v[j] W[j, o]   (v bcast as lhsT)
            pA2 = psum.tile([128, F_OUT], f32)
            nc.tensor.matmul(
                pA2[:],
                vcol[:].broadcast_to([F_IN, 128]),
                w_s[:],
                start=True,
                stop=True,
            )
            A2 = cpool.tile([128, F_OUT], bf16)
            nc.vector.tensor_copy(A2[:], pA2[:])
            if not zero_bias:
                pB2 = psum.tile([128, F_OUT], f32)
                nc.tensor.matmul(
                    pB2[:], ones_row[:], bias_s[:], start=True, stop=True
                )
                B2 = cpool.tile([128, F_OUT], bf16)
                nc.vector.tensor_copy(B2[:], pB2[:])

            # ---- out[p, n, :] = dinvS[p, n] * A2 (+ bias) ----
            # rows 0..7 on DVE (tensor_scalar 4x mode), rows 8..11 on ScalarE
            # (activation copy-with-scale); out groups ride the idle sync queue
            og_rows = [(0, 4), (4, 8), (8, 12)]
            for g, (a, b) in enumerate(og_rows):
                og = opool.tile([128, b - a, F_OUT], bf16, tag=f"og{g}",
                                name=f"og{g}")
                for j in range(b - a):
                    i = a + j
                    if zero_bias:
                        if i < DVE_ROWS:
                            nc.vector.tensor_scalar_mul(
                                og[:, j, :], A2[:], dinvSf[:, i : i + 1]
                            )
                        else:
                            nc.scalar.activation(
                                og[:, j, :],
                                A2[:],
                                mybir.ActivationFunctionType.Copy,
                                scale=dinvSf[:, i : i + 1],
                            )
                    else:
                        nc.vector.scalar_tensor_tensor(
                            og[:, j, :],
                            A2[:],
                            dinvSf[:, i : i + 1],
                            B2[:],
                            op0=mybir.AluOpType.mult,
                            op1=mybir.AluOpType.add,
                        )
                nc.sync.dma_start(out_view[:, a:b, :], og[:])

    nc.compile()
    return nc


def _get_nc(zero_bias):
    key = ("nc", zero_bias)
    if key not in _cache:
        _cache[key] = _build_nc(zero_bias)
    return _cache[key]


def _host_dinv(edge_index: np.ndarray) -> np.ndarray:
    """Exact deduplicated symmetric degree -> 1/sqrt(deg), matching
    adj[a,b]=1; adj[b,a]=1; deg=adj.sum(1)."""
    a = edge_index[0].astype(np.int64)
    b = edge_index[1].astype(np.int64)
    keys = np.unique(np.concatenate([a * N + b, b * N + a]))
    deg = np.bincount(keys // N, minlength=N).astype(np.float32)
    with np.errstate(divide="ignore"):
        dinv = (np.float32(1.0) / np.sqrt(deg)).astype(np.float32)
    return dinv


def kernel(x, edge_index, weight, bias, _trace=False):
    from concourse import bass_utils

    x = np.ascontiguousarray(x, dtype=np.float32)
    weight = np.ascontiguousarray(weight, dtype=np.float32)
    bias = np.ascontiguousarray(bias, dtype=np.float32)
    dinv = _host_dinv(np.asarray(edge_index))

    zero_bias = bool(np.all(bias == 0.0))
    nc = _get_nc(zero_bias)

    import ml_dtypes

    bf16 = ml_dtypes.bfloat16
    xp = np.zeros((N_PAD, F_IN), bf16)
    xp[:N] = x.astype(bf16)
    dp = np.zeros((N_PAD,), np.float32)
    dp[:N] = dinv
    dinvT = dp.reshape(128, R_TOT)          # dinvT[p, r] = dinv[p*96+r]

    w16 = weight.astype(bf16)
    in_maps = []
    for c in range(N_CORES):
        r0 = c * ROWS
        ds = np.zeros((ROWS_PAD,), np.float32)
        ds[:ROWS] = dinv[r0 : r0 + ROWS]
        dinvSf = np.ascontiguousarray(ds.reshape(128, NT_OUT))
        cb16 = np.ascontiguousarray(
            np.concatenate([dinvT, dinvSf], axis=1)
        ).astype(bf16)
        im = {
            "x": xp,
            "cb16": cb16,
            "dinvSf": dinvSf,
            "weight": w16,
        }
        if not zero_bias:
            im["bias"] = bias
        in_maps.append(im)

    res = bass_utils.run_bass_kernel_spmd(
        nc, in_maps, core_ids=list(range(N_CORES)), trace=_trace
    )
    out = np.concatenate(
        [
            res.results[c]["out"][:ROWS].astype(np.float32)
            for c in range(N_CORES)
        ],
        axis=0,
    )
    if _trace:
        _cache["last_results"] = res
    return out
